# revision 11
# baseline (speedup 1.0000x reference)
"""Trainium2 Bass kernel for nn_AdaptiveModalityEncoder.

Reference computation (per row r of input_data [B, D]):
    sel[r] = selection_mask[r, modality_idx] > 0.5
    out[r] = sel[r] ? gelu(x[r] @ W1 + b1) @ W2 + b2 : 0

Strategy (moe_routing, data-parallel across 8 cores):
  - Shard batch B=16384 into 8 x 2048 rows; replicate weights.
  - Host computes per-shard selected-row index lists (routing metadata);
    each core device-side: indirect-DMA gathers ONLY the selected rows,
    PE-transposes them, runs the 2-layer MLP in bf16 (fp32 accumulate),
    and indirect-DMA scatters encoded rows back into a zero-filled
    output ("mask-gather-encode-scatter", no cross-core communication).
  - Unselected rows are zero-filled by scattering from a zero tile.
  - Compute skips ~50% of rows (mask is ~Bernoulli(0.5)).

Matmul layout: activations kept feature-major (X^T tiles [D_part, rows]);
L1: H^T = W1^T @ X^T (lhsT=W1 natural). L2 flips operands so output comes
out batch-major directly: OUT = (HT)^T @ W2 (lhsT=HT slices) -> natural
rows, scattered straight to DRAM. Only the input needs a PE transpose.
"""

import sys

sys.path.insert(0, "/opt/trn_rl_repo")

import numpy as np
import ml_dtypes

# Problem constants (hardcoded per harness contract).
B, D, H, O, K = 16384, 1024, 2048, 1024, 4
NCORES = 8
R = B // NCORES  # rows per core = 2048
P = 128
# Scatter-index sentinel: one past the last valid row, skipped via
# bounds_check. Must stay small — the DGE multiplies idx by the row stride
# in 32-bit, so a huge sentinel would overflow and alias a valid row.
OOB = R

_GRAPH_CACHE = {}


def _build_graph(NG, NGZ, act="gelu"):
    """Build + compile the per-core Bass graph. NG = selected-row tiles
    (128 rows each), NGZ = zero-fill tiles. Same graph runs on all 8 cores."""
    import concourse.bass as bass
    import concourse.mybir as mybir
    import concourse.tile as tile
    from concourse import bacc
    from concourse.masks import make_identity

    f32 = mybir.dt.float32
    bf16 = mybir.dt.bfloat16
    i32 = mybir.dt.int32
    act_fn = {
        "gelu": mybir.ActivationFunctionType.Gelu_apprx_tanh,
        "tanh": mybir.ActivationFunctionType.Tanh,  # CoreSim stand-in
    }[act]

    C = NG * P  # padded selected-row count
    NCOL = 2 * NG + NGZ  # idx columns: gather | scatter | zero

    nc = bacc.Bacc("TRN2", target_bir_lowering=False, debug=False, num_devices=NCORES)

    x_d = nc.dram_tensor("x", [R, D], bf16, kind="ExternalInput")
    w1_d = nc.dram_tensor("w1", [D, H], bf16, kind="ExternalInput")
    w2_d = nc.dram_tensor("w2", [H, O], bf16, kind="ExternalInput")
    b1_d = nc.dram_tensor("b1p", [P, H // P], f32, kind="ExternalInput")
    b2_d = nc.dram_tensor("b2row", [1, O], bf16, kind="ExternalInput")
    idx_d = nc.dram_tensor("idx", [P, NCOL], i32, kind="ExternalInput")
    out_d = nc.dram_tensor("out", [R, O], f32, kind="ExternalOutput")

    KD = D // P  # 8 k-tiles for layer 1
    KH = H // P  # 16 k-tiles for layer 2
    NO = O // P  # 8 output column tiles

    # Column chunks of the gathered batch for L1 (PSUM bank = 512 fp32).
    chunks = []
    c0 = 0
    while c0 < C:
        w = min(512, C - c0)
        chunks.append((c0, w))
        c0 += w

    with tile.TileContext(nc) as tc:
        with (
            tc.tile_pool(name="w1pool", bufs=KD) as w1pool,
            tc.tile_pool(name="w2pool", bufs=KH) as w2pool,
            tc.tile_pool(name="xtp", bufs=KD) as xtp,
            tc.tile_pool(name="htp", bufs=KH) as htp,
            tc.tile_pool(name="xg", bufs=3) as xgp,
            tc.tile_pool(name="outp", bufs=2) as outp,
            tc.tile_pool(name="const", bufs=1) as constp,
            tc.tile_pool(name="pst", bufs=2, space="PSUM") as pst,  # transposes
            tc.tile_pool(name="ps1", bufs=3, space="PSUM") as ps1,  # layer 1
            tc.tile_pool(name="ps2", bufs=3, space="PSUM") as ps2,  # layer 2
        ):
            # ---- constants / small inputs ----
            # Order matters: the gpsimd (SWDGE) queue is FIFO, so the row
            # gathers must be first in line there; big memset / broadcast /
            # zero-fill work is deferred until after the gathers are issued.
            idx_sb = constp.tile([P, NCOL], i32)
            nc.sync.dma_start(idx_sb[:], idx_d[:])
            b1_sb = constp.tile([P, H // P], f32)
            nc.sync.dma_start(b1_sb[:], b1_d[:])
            b2_sb = constp.tile([1, O], bf16)
            nc.sync.dma_start(b2_sb[:], b2_d[:])
            ident = constp.tile([P, P], bf16)
            nc.vector.memset(ident[:], 0.0)
            make_identity(nc, ident, nomemset=True)

            xt_sb = [
                xtp.tile([P, C], bf16, tag="xt", name=f"xtsb{j}") for j in range(KD)
            ]
            ht_sb = [
                htp.tile([P, C], bf16, tag="ht", name=f"htsb{h}") for h in range(KH)
            ]
            w1_sb = [
                w1pool.tile([P, H], bf16, tag="w1", name=f"w1sb{k}")
                for k in range(KD)
            ]
            w2_sb = [
                w2pool.tile([P, O], bf16, tag="w2", name=f"w2sb{k}")
                for k in range(KH)
            ]

            def gather_and_transpose(g):
                xg = xgp.tile([P, D], bf16, tag="xg", name=f"xg{g}")
                nc.gpsimd.indirect_dma_start(
                    out=xg[:],
                    out_offset=None,
                    in_=x_d[:],
                    in_offset=bass.IndirectOffsetOnAxis(
                        ap=idx_sb[:, g : g + 1], axis=0
                    ),
                )
                for j in range(KD):
                    tp = pst.tile([P, P], bf16, tag="tp", name=f"tp{g}_{j}")
                    nc.tensor.transpose(tp[:], xg[:, j * P : (j + 1) * P], ident[:])
                    nc.vector.tensor_copy(
                        xt_sb[j][:, g * P : (g + 1) * P], tp[:]
                    )

            def l1_chunk(c0, cw):
                for h in range(KH):
                    acc = ps1.tile([P, 512], f32, tag="l1acc", name=f"l1acc{c0}_{h}")
                    for k in range(KD):
                        nc.tensor.matmul(
                            acc[:, :cw],
                            w1_sb[k][:, h * P : (h + 1) * P],
                            xt_sb[k][:, c0 : c0 + cw],
                            start=(k == 0),
                            stop=(k == KD - 1),
                        )
                    nc.scalar.activation(
                        ht_sb[h][:, c0 : c0 + cw],
                        acc[:, :cw],
                        act_fn,
                        bias=b1_sb[:, h : h + 1],
                    )

            # Chunk 0's gathers, then W1 (sync queue), then remaining gathers.
            g_hi0 = min(-(-chunks[0][1] // P), NG)
            for g in range(g_hi0):
                gather_and_transpose(g)
            for k in range(KD):
                nc.sync.dma_start(w1_sb[k][:], w1_d[k * P : (k + 1) * P, :])

            # Layer 1 chunk 0 as early as possible; the remaining gathers'
            # DMAs drain while chunk 0's matmuls run. W2 rides behind W1.
            l1_chunk(*chunks[0])
            for g in range(g_hi0, NG):
                gather_and_transpose(g)
            for k in range(KH):
                nc.sync.dma_start(w2_sb[k][:], w2_d[k * P : (k + 1) * P, :])

            # Zero-fill unselected rows — gpsimd queue is free now; these
            # writes overlap layer-1 compute.
            zero_sb = constp.tile([P, O], f32)
            nc.vector.memset(zero_sb[:], 0.0)
            b2_rep = constp.tile([P, O], bf16)
            nc.gpsimd.partition_broadcast(b2_rep[:], b2_sb[:])
            for z in range(NGZ):
                nc.gpsimd.indirect_dma_start(
                    out=out_d[:],
                    out_offset=bass.IndirectOffsetOnAxis(
                        ap=idx_sb[:, 2 * NG + z : 2 * NG + z + 1], axis=0
                    ),
                    in_=zero_sb[:],
                    in_offset=None,
                    bounds_check=R - 1,
                    oob_is_err=False,
                )

            for c0, cw in chunks[1:]:
                l1_chunk(c0, cw)

            # ---- layer 2: OUT[c-rows, o] = (H^T)^T @ W2 + b2, batch-major ----
            for r in range(NG):
                out_sb = outp.tile([P, O], f32)
                for oc in range(2):
                    acc = ps2.tile([P, 512], f32)
                    for k in range(KH):
                        nc.tensor.matmul(
                            acc[:],
                            ht_sb[k][:, r * P : (r + 1) * P],
                            w2_sb[k][:, oc * 512 : (oc + 1) * 512],
                            start=(k == 0),
                            stop=(k == KH - 1),
                        )
                    nc.vector.tensor_add(
                        out_sb[:, oc * 512 : (oc + 1) * 512],
                        acc[:],
                        b2_rep[:, oc * 512 : (oc + 1) * 512],
                    )
                nc.gpsimd.indirect_dma_start(
                    out=out_d[:],
                    out_offset=bass.IndirectOffsetOnAxis(
                        ap=idx_sb[:, NG + r : NG + r + 1], axis=0
                    ),
                    in_=out_sb[:],
                    in_offset=None,
                    bounds_check=R - 1,
                    oob_is_err=False,
                )

    nc.compile()
    return nc


def _get_graph(NG, NGZ, act="gelu"):
    key = (NG, NGZ, act)
    if key not in _GRAPH_CACHE:
        _GRAPH_CACHE[key] = _build_graph(NG, NGZ, act)
    return _GRAPH_CACHE[key]


def _pack_idx(rows, n_tiles, pad_mode):
    """rows -> [128, n_tiles] int32, column g = rows[g*128:(g+1)*128]."""
    cap = n_tiles * P
    if pad_mode == "dup":
        pad = rows[-1] if len(rows) else 0
    else:
        pad = OOB
    padded = np.full(cap, pad, dtype=np.int64)
    padded[: len(rows)] = rows
    return np.ascontiguousarray(padded.reshape(n_tiles, P).T).astype(np.int32)


def prepare(input_data, selection_mask, W1, b1, W2, b2, modality_idx, act="gelu"):
    """Host-side sharding/routing prep. Returns (nc, in_maps) or None if no
    rows are selected (output is all zeros)."""
    x = np.asarray(input_data, dtype=np.float32)
    mask = np.asarray(selection_mask, dtype=np.float32)
    midx = int(np.asarray(modality_idx))
    sel = mask[:, midx] > 0.5

    sel_rows = [np.nonzero(sel[i * R : (i + 1) * R])[0] for i in range(NCORES)]
    unsel_rows = [np.nonzero(~sel[i * R : (i + 1) * R])[0] for i in range(NCORES)]
    max_sel = max(len(r) for r in sel_rows)
    max_unsel = max(len(r) for r in unsel_rows)
    if max_sel == 0:
        return None
    NG = -(-max_sel // P)
    NGZ = -(-max_unsel // P)

    nc = _get_graph(NG, NGZ, act)

    bf = ml_dtypes.bfloat16
    w1_b = np.asarray(W1, dtype=np.float32).astype(bf)
    w2_b = np.asarray(W2, dtype=np.float32).astype(bf)
    b1p = np.ascontiguousarray(
        np.asarray(b1, dtype=np.float32).reshape(H // P, P).T
    )
    b2row = np.asarray(b2, dtype=np.float32).reshape(1, O).astype(bf)

    in_maps = []
    for i in range(NCORES):
        idx = np.concatenate(
            [
                _pack_idx(sel_rows[i], NG, "dup"),
                _pack_idx(sel_rows[i], NG, "oob"),
                _pack_idx(unsel_rows[i], NGZ, "oob"),
            ],
            axis=1,
        )
        in_maps.append(
            {
                "x": x[i * R : (i + 1) * R].astype(bf),
                "w1": w1_b,
                "w2": w2_b,
                "b1p": b1p,
                "b2row": b2row,
                "idx": idx,
            }
        )
    return nc, in_maps


def kernel(input_data, selection_mask, W1, b1, W2, b2, modality_idx):
    prep = prepare(input_data, selection_mask, W1, b1, W2, b2, modality_idx)
    if prep is None:
        return np.zeros((B, O), dtype=np.float32)
    nc, in_maps = prep

    from concourse.bass_utils import run_bass_kernel_spmd

    res = run_bass_kernel_spmd(nc, in_maps, core_ids=list(range(NCORES)))
    out = np.concatenate([res.results[i]["out"] for i in range(NCORES)], axis=0)
    return np.ascontiguousarray(out.astype(np.float32))


# revision 13
# speedup vs baseline: 1.0350x; 1.0350x over previous
"""Trainium2 Bass kernel for nn_AdaptiveModalityEncoder.

Reference computation (per row r of input_data [B, D]):
    sel[r] = selection_mask[r, modality_idx] > 0.5
    out[r] = sel[r] ? gelu(x[r] @ W1 + b1) @ W2 + b2 : 0

Strategy (moe_routing, data-parallel across 8 cores):
  - Shard batch B=16384 into 8 x 2048 rows; replicate weights.
  - Host computes per-shard selected-row index lists (routing metadata);
    each core device-side: indirect-DMA gathers ONLY the selected rows,
    PE-transposes them, runs the 2-layer MLP in bf16 (fp32 accumulate),
    and indirect-DMA scatters encoded rows back into a zero-filled
    output ("mask-gather-encode-scatter", no cross-core communication).
  - Unselected rows are zero-filled by scattering from a zero tile.
  - Compute skips ~50% of rows (mask is ~Bernoulli(0.5)).

Matmul layout: activations kept feature-major (X^T tiles [D_part, rows]);
L1: H^T = W1^T @ X^T (lhsT=W1 natural). L2 flips operands so output comes
out batch-major directly: OUT = (HT)^T @ W2 (lhsT=HT slices) -> natural
rows, scattered straight to DRAM. Only the input needs a PE transpose.
"""

import sys

sys.path.insert(0, "/opt/trn_rl_repo")

import numpy as np
import ml_dtypes

# Problem constants (hardcoded per harness contract).
B, D, H, O, K = 16384, 1024, 2048, 1024, 4
NCORES = 8
R = B // NCORES  # rows per core = 2048
P = 128
# Scatter-index sentinel: one past the last valid row, skipped via
# bounds_check. Must stay small — the DGE multiplies idx by the row stride
# in 32-bit, so a huge sentinel would overflow and alias a valid row.
OOB = R

_GRAPH_CACHE = {}


def _build_graph(NG, NGZ, act="gelu"):
    """Build + compile the per-core Bass graph. NG = selected-row tiles
    (128 rows each), NGZ = zero-fill tiles. Same graph runs on all 8 cores."""
    import concourse.bass as bass
    import concourse.mybir as mybir
    import concourse.tile as tile
    from concourse import bacc
    from concourse.masks import make_identity

    f32 = mybir.dt.float32
    bf16 = mybir.dt.bfloat16
    i32 = mybir.dt.int32
    act_fn = {
        "gelu": mybir.ActivationFunctionType.Gelu_apprx_tanh,
        "tanh": mybir.ActivationFunctionType.Tanh,  # CoreSim stand-in
    }[act]

    C = NG * P  # padded selected-row count
    NCOL = 2 * NG + NGZ  # idx columns: gather | scatter | zero

    nc = bacc.Bacc("TRN2", target_bir_lowering=False, debug=False, num_devices=NCORES)

    x_d = nc.dram_tensor("x", [R, D], bf16, kind="ExternalInput")
    w1_d = nc.dram_tensor("w1", [D, H], bf16, kind="ExternalInput")
    w2_d = nc.dram_tensor("w2", [H, O], bf16, kind="ExternalInput")
    b1_d = nc.dram_tensor("b1p", [P, H // P], f32, kind="ExternalInput")
    b2_d = nc.dram_tensor("b2row", [1, O], bf16, kind="ExternalInput")
    idx_d = nc.dram_tensor("idx", [P, NCOL], i32, kind="ExternalInput")
    out_d = nc.dram_tensor("out", [R, O], f32, kind="ExternalOutput")

    KD = D // P  # 8 k-tiles for layer 1
    KH = H // P  # 16 k-tiles for layer 2
    NO = O // P  # 8 output column tiles

    # Column chunks of the gathered batch for L1 (PSUM bank = 512 fp32).
    chunks = []
    c0 = 0
    while c0 < C:
        w = min(512, C - c0)
        chunks.append((c0, w))
        c0 += w

    with tile.TileContext(nc) as tc:
        with (
            tc.tile_pool(name="w1pool", bufs=KD) as w1pool,
            tc.tile_pool(name="w2pool", bufs=KH) as w2pool,
            tc.tile_pool(name="xtp", bufs=2 * KD) as xtp,
            tc.tile_pool(name="htp", bufs=2 * KH) as htp,
            tc.tile_pool(name="outp", bufs=2) as outp,
            tc.tile_pool(name="const", bufs=1) as constp,
            tc.tile_pool(name="pst", bufs=2, space="PSUM") as pst,  # transposes
            tc.tile_pool(name="ps1", bufs=3, space="PSUM") as ps1,  # layer 1
            tc.tile_pool(name="ps2", bufs=3, space="PSUM") as ps2,  # layer 2
        ):
            # ---- constants / small inputs ----
            # The gpsimd (SWDGE) queue is FIFO: row gathers go first there;
            # zero-fill and broadcasts are deferred until the gathers issued.
            idx_sb = constp.tile([P, NCOL], i32)
            nc.sync.dma_start(idx_sb[:], idx_d[:])
            b1_sb = constp.tile([P, H // P], f32)
            nc.sync.dma_start(b1_sb[:], b1_d[:])
            b2_sb = constp.tile([1, O], bf16)
            nc.sync.dma_start(b2_sb[:], b2_d[:])
            ident = constp.tile([P, P], bf16)
            nc.vector.memset(ident[:], 0.0)
            make_identity(nc, ident, nomemset=True)

            # ---- gather all selected rows into one landing tile ----
            # A single wide tile avoids staging-slot starvation: all NG
            # indirect DMAs stream back-to-back on the gpsimd queue while
            # the PE consumes earlier columns.
            xg_all = constp.tile([P, NG * D], bf16)
            for g in range(NG):
                nc.gpsimd.indirect_dma_start(
                    out=xg_all[:, g * D : (g + 1) * D],
                    out_offset=None,
                    in_=x_d[:],
                    in_offset=bass.IndirectOffsetOnAxis(
                        ap=idx_sb[:, g : g + 1], axis=0
                    ),
                )

            # ---- weights (resident, bf16) ----
            w1_sb = [
                w1pool.tile([P, H], bf16, tag="w1", name=f"w1sb{k}")
                for k in range(KD)
            ]
            for k in range(KD):
                nc.sync.dma_start(w1_sb[k][:], w1_d[k * P : (k + 1) * P, :])
            w2_sb = [
                w2pool.tile([P, O], bf16, tag="w2", name=f"w2sb{k}")
                for k in range(KH)
            ]
            for k in range(KH):
                nc.sync.dma_start(w2_sb[k][:], w2_d[k * P : (k + 1) * P, :])

            # Chunks of whole gather-tiles (up to 3 x 128 = 384 columns).
            GPC = 3  # gather tiles per chunk
            gchunks = [(g0, min(GPC, NG - g0)) for g0 in range(0, NG, GPC)]

            first = True
            for g0, ng in gchunks:
                cw = ng * P
                # transpose chunk columns into X^T layout
                xt_c = [
                    xtp.tile([P, GPC * P], bf16, tag="xt", name=f"xt{g0}_{j}")
                    for j in range(KD)
                ]
                for gl in range(ng):
                    for j in range(KD):
                        tp = pst.tile([P, P], bf16, tag="tp", name=f"tp{g0+gl}_{j}")
                        nc.tensor.transpose(
                            tp[:],
                            xg_all[:, (g0 + gl) * D + j * P : (g0 + gl) * D + (j + 1) * P],
                            ident[:],
                        )
                        nc.vector.tensor_copy(
                            xt_c[j][:, gl * P : (gl + 1) * P], tp[:]
                        )

                # layer 1: H^T chunk = gelu(W1^T @ X^T + b1)
                ht_c = [
                    htp.tile([P, GPC * P], bf16, tag="ht", name=f"ht{g0}_{h}")
                    for h in range(KH)
                ]
                for h in range(KH):
                    acc = ps1.tile([P, GPC * P], f32, tag="l1acc", name=f"l1a{g0}_{h}")
                    for k in range(KD):
                        nc.tensor.matmul(
                            acc[:, :cw],
                            w1_sb[k][:, h * P : (h + 1) * P],
                            xt_c[k][:, :cw],
                            start=(k == 0),
                            stop=(k == KD - 1),
                        )
                    nc.scalar.activation(
                        ht_c[h][:, :cw],
                        acc[:, :cw],
                        act_fn,
                        bias=b1_sb[:, h : h + 1],
                    )

                if first:
                    # gpsimd queue has drained the gathers by now: issue the
                    # zero-fill of unselected rows; overlaps layer-1 compute.
                    first = False
                    zero_sb = constp.tile([P, O], f32)
                    nc.vector.memset(zero_sb[:], 0.0)
                    b2_rep = constp.tile([P, O], bf16)
                    nc.gpsimd.partition_broadcast(b2_rep[:], b2_sb[:])
                    for z in range(NGZ):
                        nc.gpsimd.indirect_dma_start(
                            out=out_d[:],
                            out_offset=bass.IndirectOffsetOnAxis(
                                ap=idx_sb[:, 2 * NG + z : 2 * NG + z + 1], axis=0
                            ),
                            in_=zero_sb[:],
                            in_offset=None,
                            bounds_check=R - 1,
                            oob_is_err=False,
                        )

                # layer 2 for this chunk's row tiles, batch-major, + scatter
                for rl in range(ng):
                    r = g0 + rl
                    out_sb = outp.tile([P, O], f32, tag="outsb", name=f"osb{r}")
                    for oc in range(2):
                        acc2 = ps2.tile([P, 512], f32, tag="l2acc", name=f"l2a{r}_{oc}")
                        for k in range(KH):
                            nc.tensor.matmul(
                                acc2[:],
                                ht_c[k][:, rl * P : (rl + 1) * P],
                                w2_sb[k][:, oc * 512 : (oc + 1) * 512],
                                start=(k == 0),
                                stop=(k == KH - 1),
                            )
                        nc.vector.tensor_add(
                            out_sb[:, oc * 512 : (oc + 1) * 512],
                            acc2[:],
                            b2_rep[:, oc * 512 : (oc + 1) * 512],
                        )
                    nc.gpsimd.indirect_dma_start(
                        out=out_d[:],
                        out_offset=bass.IndirectOffsetOnAxis(
                            ap=idx_sb[:, NG + r : NG + r + 1], axis=0
                        ),
                        in_=out_sb[:],
                        in_offset=None,
                        bounds_check=R - 1,
                        oob_is_err=False,
                    )

    nc.compile()
    return nc


def _get_graph(NG, NGZ, act="gelu"):
    key = (NG, NGZ, act)
    if key not in _GRAPH_CACHE:
        _GRAPH_CACHE[key] = _build_graph(NG, NGZ, act)
    return _GRAPH_CACHE[key]


def _pack_idx(rows, n_tiles, pad_mode):
    """rows -> [128, n_tiles] int32, column g = rows[g*128:(g+1)*128]."""
    cap = n_tiles * P
    if pad_mode == "dup":
        pad = rows[-1] if len(rows) else 0
    else:
        pad = OOB
    padded = np.full(cap, pad, dtype=np.int64)
    padded[: len(rows)] = rows
    return np.ascontiguousarray(padded.reshape(n_tiles, P).T).astype(np.int32)


def prepare(input_data, selection_mask, W1, b1, W2, b2, modality_idx, act="gelu"):
    """Host-side sharding/routing prep. Returns (nc, in_maps) or None if no
    rows are selected (output is all zeros)."""
    x = np.asarray(input_data, dtype=np.float32)
    mask = np.asarray(selection_mask, dtype=np.float32)
    midx = int(np.asarray(modality_idx))
    sel = mask[:, midx] > 0.5

    sel_rows = [np.nonzero(sel[i * R : (i + 1) * R])[0] for i in range(NCORES)]
    unsel_rows = [np.nonzero(~sel[i * R : (i + 1) * R])[0] for i in range(NCORES)]
    max_sel = max(len(r) for r in sel_rows)
    max_unsel = max(len(r) for r in unsel_rows)
    if max_sel == 0:
        return None
    NG = -(-max_sel // P)
    NGZ = -(-max_unsel // P)

    nc = _get_graph(NG, NGZ, act)

    bf = ml_dtypes.bfloat16
    w1_b = np.asarray(W1, dtype=np.float32).astype(bf)
    w2_b = np.asarray(W2, dtype=np.float32).astype(bf)
    b1p = np.ascontiguousarray(
        np.asarray(b1, dtype=np.float32).reshape(H // P, P).T
    )
    b2row = np.asarray(b2, dtype=np.float32).reshape(1, O).astype(bf)

    in_maps = []
    for i in range(NCORES):
        idx = np.concatenate(
            [
                _pack_idx(sel_rows[i], NG, "dup"),
                _pack_idx(sel_rows[i], NG, "oob"),
                _pack_idx(unsel_rows[i], NGZ, "oob"),
            ],
            axis=1,
        )
        in_maps.append(
            {
                "x": x[i * R : (i + 1) * R].astype(bf),
                "w1": w1_b,
                "w2": w2_b,
                "b1p": b1p,
                "b2row": b2row,
                "idx": idx,
            }
        )
    return nc, in_maps


def kernel(input_data, selection_mask, W1, b1, W2, b2, modality_idx):
    prep = prepare(input_data, selection_mask, W1, b1, W2, b2, modality_idx)
    if prep is None:
        return np.zeros((B, O), dtype=np.float32)
    nc, in_maps = prep

    from concourse.bass_utils import run_bass_kernel_spmd

    res = run_bass_kernel_spmd(nc, in_maps, core_ids=list(range(NCORES)))
    out = np.concatenate([res.results[i]["out"] for i in range(NCORES)], axis=0)
    return np.ascontiguousarray(out.astype(np.float32))
